# revision 1
# baseline (speedup 1.0000x reference)
"""DeepseekECMoE (expert-choice MoE) Trainium2 kernel, 8-way expert-parallel.

Layout per core c (SPMD, differences only via inputs):
  - routed expert c for all 8 batches: gate (f32r matmul) -> softmax over E
    (DVE tree) -> exact top-256 per (b, e=c) via max8/max_index/match_replace
    -> dispatch via one-hot matmul (bf16) -> expert MLP (bf16 matmuls, exact
    erf-gelu on ACT) -> unweighted token outputs + scores + indices out.
  - shared expert for batch b=c (bf16 matmuls).
Host combines: scatter-add weighted expert outputs, transpose, add shared.
"""
import numpy as np
import ml_dtypes

import concourse.bass as bass
import concourse.tile as tile
from concourse import bacc, mybir
from concourse.bass2jax import install_neuronx_cc_hook, _bass_exec_p, partition_id_tensor

B, S, H, E = 8, 1024, 1024, 8
I, ISH, CAP = 2048, 2048, 256
P = 128
HC, SC, NI, NISH = H // P, S // P, I // P, ISH // P
N_CORES = 8
dt = mybir.dt
BF16 = ml_dtypes.bfloat16

_CACHE: dict = {}


def _build_nc(act_name="Gelu"):
    nc = bacc.Bacc("TRN2", target_bir_lowering=False, debug=False,
                   num_devices=N_CORES)

    # ---- DRAM I/O ----
    hidT = nc.dram_tensor("hidT", [B, H, S], dt.float32r, kind="ExternalInput")
    hidb = nc.dram_tensor("hidb", [B, S, H], dt.bfloat16, kind="ExternalInput")
    gw = nc.dram_tensor("gw", [P, HC * E], dt.float32r, kind="ExternalInput")
    esel = nc.dram_tensor("esel", [E, 1], dt.float32r, kind="ExternalInput")
    ones8 = nc.dram_tensor("ones8", [E, 1], dt.float32r, kind="ExternalInput")
    bsel = nc.dram_tensor("bsel", [E, E * P], dt.float32r, kind="ExternalInput")
    gut = nc.dram_tensor("gut", [2, NI, P, HC * P], dt.bfloat16, kind="ExternalInput")
    dpTb = nc.dram_tensor("dpTb", [I, H], dt.bfloat16, kind="ExternalInput")
    sgut = nc.dram_tensor("sgut", [2, NISH, P, HC * P], dt.bfloat16, kind="ExternalInput")
    hshb = nc.dram_tensor("hshb", [H, S], dt.bfloat16, kind="ExternalInput")
    sdTb = nc.dram_tensor("sdTb", [ISH, H], dt.bfloat16, kind="ExternalInput")

    w_out = nc.dram_tensor("w_out", [B, CAP, H], dt.float32, kind="ExternalOutput")
    scoreso = nc.dram_tensor("scoreso", [B, CAP], dt.float32, kind="ExternalOutput")
    idxo = nc.dram_tensor("idxo", [B, CAP], dt.uint32, kind="ExternalOutput")
    sh_out = nc.dram_tensor("sh_out", [S, H], dt.float32, kind="ExternalOutput")

    AF = mybir.ActivationFunctionType
    ACT = getattr(AF, act_name)
    from contextlib import ExitStack
    with tile.TileContext(nc) as tc:
        with ExitStack() as ctx:
            pool = lambda name, bufs, **kw: ctx.enter_context(
                tc.tile_pool(name=name, bufs=bufs, **kw))
            pconst = pool("consts", 1)
            phtstr = pool("htstr", 3)
            pexp = pool("exp", 2)
            pwork = pool("work", 1)
            prden = pool("rden", 1)
            proute = pool("route", 1)
            phsh = pool("hsh", 8)
            psw = pool("sw", 4)
            pactsh = pool("actsh", 16)
            pdstr = pool("dstr", 17)
            pactT = pool("actT", 16)
            ptok = pool("tok", 9)
            pM = pool("Mpool", 8)
            phstr = pool("hstr", 9)
            pguw = pool("guw", 4)
            pgel = pool("gel", 2)
            pwo = pool("wo", 3)
            psmall = pool("small", 2)
            pgu = pool("pgu", 2, space="PSUM")
            pdown = pool("pdown", 2, space="PSUM")
            ptokp = pool("ptokp", 2, space="PSUM")
            # ---- constants ----
            t_gw = pconst.tile([P, HC * E], dt.float32r)
            nc.sync.dma_start(t_gw[:], gw[:])
            t_esel = pconst.tile([E, 1], dt.float32r)
            nc.sync.dma_start(t_esel[:], esel[:])
            t_ones8 = pconst.tile([E, 1], dt.float32r)
            nc.sync.dma_start(t_ones8[:], ones8[:])
            t_bsel = pconst.tile([E, E * P], dt.float32r)
            nc.sync.dma_start(t_bsel[:], bsel[:])
            t_iot = pconst.tile([P, SC], dt.int32)
            nc.gpsimd.iota(t_iot[:], pattern=[[P, SC]], base=0, channel_multiplier=1)
            t_iotf = pconst.tile([P, SC], dt.float32)
            nc.vector.tensor_copy(t_iotf[:], t_iot[:])

            # ---- gate + routing ----
            afftile = proute.tile([E, S], dt.float32)
            t_scores = proute.tile([E, CAP], dt.float32)
            t_idxu = proute.tile([E, CAP], dt.uint32)
            t_idxf = proute.tile([E, CAP], dt.float32)
            t_idxfr = proute.tile([E, CAP], dt.float32r)

            for b in range(B):
                exp_b = pexp.tile([E, S], dt.float32r)
                for sblk in range(2):
                    pl = ptokp.tile([E, 512], dt.float32, tag="ptk", name="pl")
                    for hc in range(HC):
                        ht = phtstr.tile([P, 512], dt.float32r)
                        nc.sync.dma_start(
                            ht[:], hidT[b, hc * P:(hc + 1) * P,
                                        sblk * 512:(sblk + 1) * 512])
                        nc.tensor.matmul(pl[:], t_gw[:, hc * E:(hc + 1) * E],
                                         ht[:], start=(hc == 0), stop=(hc == HC - 1))
                    nc.scalar.activation(exp_b[:, sblk * 512:(sblk + 1) * 512],
                                         pl[:], AF.Exp)
                rden = prden.tile([1, S], dt.float32)
                affrow = pwork.tile([1, S], dt.float32, tag="rt", name="affrow")
                for sblk in range(2):
                    sl = slice(sblk * 512, (sblk + 1) * 512)
                    pden = ptokp.tile([1, 512], dt.float32, tag="ptk", name="pden")
                    nc.tensor.matmul(pden[:], t_ones8[:], exp_b[:, sl],
                                     start=True, stop=True)
                    nc.vector.reciprocal(rden[:, sl], pden[:])
                    psel = ptokp.tile([1, 512], dt.float32, tag="ptk", name="psel")
                    nc.tensor.matmul(psel[:], t_esel[:], exp_b[:, sl],
                                     start=True, stop=True)
                    nc.vector.tensor_mul(affrow[:, sl], psel[:], rden[:, sl])
                nc.sync.dma_start(afftile[b:b + 1, :], affrow[:])

            for i in range(CAP // 8):
                sc8 = t_scores[:, i * 8:(i + 1) * 8]
                nc.vector.max(sc8, afftile[:])
                nc.vector.max_index(t_idxu[:, i * 8:(i + 1) * 8], sc8, afftile[:])
                nc.vector.match_replace(afftile[:], sc8, afftile[:], -1e30)
            nc.sync.dma_start(scoreso[:], t_scores[:])
            nc.sync.dma_start(idxo[:], t_idxu[:])
            nc.vector.tensor_copy(t_idxf[:], t_idxu[:])
            nc.vector.tensor_copy(t_idxfr[:], t_idxf[:])

            # ---- shared expert (batch c fed via hshb) ----
            hsh = []
            for hc in range(HC):
                t = phsh.tile([P, S], dt.bfloat16, tag="hsh", name="hsh")
                nc.sync.dma_start(t[:], hshb[hc * P:(hc + 1) * P, :])
                hsh.append(t)
            actsh = []
            for i in range(NISH):
                sg = psw.tile([P, HC * P], dt.bfloat16, bufs=2)
                nc.sync.dma_start(sg[:], sgut[0, i])
                su = psw.tile([P, HC * P], dt.bfloat16, bufs=2)
                nc.sync.dma_start(su[:], sgut[1, i])
                a = pactsh.tile([P, S], dt.bfloat16)
                for sblk in range(2):
                    pg = pgu.tile([P, 512], dt.float32, tag="pg", name="pg", bufs=2)
                    for hc in range(HC):
                        nc.tensor.matmul(pg[:], sg[:, hc * P:(hc + 1) * P],
                                         hsh[hc][:, sblk * 512:(sblk + 1) * 512],
                                         start=(hc == 0), stop=(hc == HC - 1))
                    pu = pgu.tile([P, 512], dt.float32, tag="pu", name="pu", bufs=2)
                    for hc in range(HC):
                        nc.tensor.matmul(pu[:], su[:, hc * P:(hc + 1) * P],
                                         hsh[hc][:, sblk * 512:(sblk + 1) * 512],
                                         start=(hc == 0), stop=(hc == HC - 1))
                    gel = pgel.tile([P, 512], dt.float32)
                    nc.scalar.activation(gel[:], pg[:], ACT)
                    nc.vector.tensor_mul(a[:, sblk * 512:(sblk + 1) * 512],
                                         gel[:], pu[:])
                actsh.append(a)
            sdt = []
            for ic in range(NISH):
                t = pdstr.tile([P, H], dt.bfloat16, tag="dstr", name="dstr")
                nc.sync.dma_start(t[:], sdTb[ic * P:(ic + 1) * P, :])
                sdt.append(t)
            for sblk in range(SC):
                for hh in range(2):
                    pd = pdown.tile([P, 512], dt.float32)
                    for ic in range(NISH):
                        nc.tensor.matmul(pd[:],
                                         actsh[ic][:, sblk * P:(sblk + 1) * P],
                                         sdt[ic][:, hh * 512:(hh + 1) * 512],
                                         start=(ic == 0), stop=(ic == NISH - 1))
                    sho = pwo.tile([P, 512], dt.float32, tag="wo", name="wo")
                    nc.scalar.copy(sho[:], pd[:])
                    nc.sync.dma_start(
                        sh_out[sblk * P:(sblk + 1) * P, hh * 512:(hh + 1) * 512],
                        sho[:])

            # ---- routed expert, batch pairs ----
            for pair in range(B // 2):
                b0 = 2 * pair
                tokT = []
                for hc in range(HC):
                    tokT.append(ptok.tile([P, 2 * CAP], dt.bfloat16, tag="tokT", name="tokT"))
                for bi in range(2):
                    b = b0 + bi
                    pib = ptokp.tile([P, CAP], dt.float32, tag="ptk", name="pib")
                    nc.tensor.matmul(pib[:], t_bsel[:, b * P:(b + 1) * P],
                                     t_idxfr[:], start=True, stop=True)
                    idxB = psmall.tile([P, CAP], dt.float32)
                    nc.vector.tensor_copy(idxB[:], pib[:])
                    Ms = []
                    for sc in range(SC):
                        m = pM.tile([P, CAP], dt.bfloat16, tag="M", name="M")
                        nc.vector.tensor_scalar(m[:], idxB[:], t_iotf[:, sc:sc + 1],
                                                None, mybir.AluOpType.is_equal)
                        Ms.append(m)
                    hh_tiles = []
                    for sc in range(SC):
                        t = phstr.tile([P, H], dt.bfloat16, tag="hstr", name="hstr")
                        nc.sync.dma_start(t[:], hidb[b, sc * P:(sc + 1) * P, :])
                        hh_tiles.append(t)
                    for hblk in range(HC):
                        pt = ptokp.tile([P, CAP], dt.float32, tag="ptk", name="pt")
                        for sc in range(SC):
                            nc.tensor.matmul(pt[:],
                                             hh_tiles[sc][:, hblk * P:(hblk + 1) * P],
                                             Ms[sc][:],
                                             start=(sc == 0), stop=(sc == SC - 1))
                        nc.vector.tensor_copy(
                            tokT[hblk][:, bi * CAP:(bi + 1) * CAP], pt[:])

                actT = []
                for i in range(NI):
                    sg = pguw.tile([P, HC * P], dt.bfloat16, bufs=2)
                    nc.sync.dma_start(sg[:], gut[0, i])
                    su = pguw.tile([P, HC * P], dt.bfloat16, bufs=2)
                    nc.sync.dma_start(su[:], gut[1, i])
                    pg = pgu.tile([P, 2 * CAP], dt.float32, tag="pg", name="pg", bufs=2)
                    for hc in range(HC):
                        nc.tensor.matmul(pg[:], sg[:, hc * P:(hc + 1) * P],
                                         tokT[hc][:],
                                         start=(hc == 0), stop=(hc == HC - 1))
                    pu = pgu.tile([P, 2 * CAP], dt.float32, tag="pu", name="pu", bufs=2)
                    for hc in range(HC):
                        nc.tensor.matmul(pu[:], su[:, hc * P:(hc + 1) * P],
                                         tokT[hc][:],
                                         start=(hc == 0), stop=(hc == HC - 1))
                    gel = pgel.tile([P, 2 * CAP], dt.float32)
                    nc.scalar.activation(gel[:], pg[:], ACT)
                    a = pactT.tile([P, 2 * CAP], dt.bfloat16)
                    nc.vector.tensor_mul(a[:], gel[:], pu[:])
                    actT.append(a)

                dpt = []
                for ic in range(NI):
                    t = pdstr.tile([P, H], dt.bfloat16, tag="dstr", name="dstr")
                    nc.sync.dma_start(t[:], dpTb[ic * P:(ic + 1) * P, :])
                    dpt.append(t)
                for tb in range(4):
                    b = b0 + tb // 2
                    rblk = tb % 2
                    for hh in range(2):
                        pd = pdown.tile([P, 512], dt.float32)
                        for ic in range(NI):
                            nc.tensor.matmul(pd[:],
                                             actT[ic][:, tb * P:(tb + 1) * P],
                                             dpt[ic][:, hh * 512:(hh + 1) * 512],
                                             start=(ic == 0), stop=(ic == NI - 1))
                        wo = pwo.tile([P, 512], dt.float32, tag="wo", name="wo")
                        nc.scalar.copy(wo[:], pd[:])
                        nc.sync.dma_start(
                            w_out[b, rblk * P:(rblk + 1) * P,
                                  hh * 512:(hh + 1) * 512], wo[:])

    nc.compile()
    return nc


class _Exec:
    """Cached multi-core PJRT executor (mirrors bass2jax.run_bass_via_pjrt)."""

    def __init__(self, nc):
        import jax
        from jax.sharding import Mesh, PartitionSpec
        from jax.experimental.shard_map import shard_map

        install_neuronx_cc_hook()
        self.nc = nc
        in_names, out_names, out_avals = [], [], []
        partition_name = (nc.partition_id_tensor.name
                          if nc.partition_id_tensor else None)
        for alloc in nc.m.functions[0].allocations:
            if not isinstance(alloc, mybir.MemoryLocationSet):
                continue
            name = alloc.memorylocations[0].name
            if alloc.kind == "ExternalInput":
                if name != partition_name:
                    in_names.append(name)
            elif alloc.kind == "ExternalOutput":
                out_names.append(name)
                out_avals.append(jax.core.ShapedArray(
                    tuple(alloc.tensor_shape), mybir.dt.np(alloc.dtype)))
        self.in_names, self.out_names, self.out_avals = in_names, out_names, out_avals
        self.partition_name = partition_name
        n_params = len(in_names)
        n_outs = len(out_names)
        all_in_names = list(in_names) + list(out_names)
        if partition_name is not None:
            all_in_names.append(partition_name)

        def _body(*args):
            operands = list(args)
            if partition_name is not None:
                operands.append(partition_id_tensor())
            outs = _bass_exec_p.bind(
                *operands,
                out_avals=tuple(out_avals),
                in_names=tuple(all_in_names),
                out_names=tuple(out_names),
                lowering_input_output_aliases=(),
                sim_require_finite=True,
                sim_require_nnan=True,
                nc=nc,
            )
            return tuple(outs)

        devices = jax.devices()[:N_CORES]
        mesh = Mesh(np.asarray(devices), ("core",))
        in_specs = (PartitionSpec("core"),) * (n_params + n_outs)
        out_specs = (PartitionSpec("core"),) * n_outs
        self.sharded = jax.jit(
            shard_map(_body, mesh=mesh, in_specs=in_specs, out_specs=out_specs,
                      check_rep=False),
            donate_argnums=tuple(range(n_params, n_params + n_outs)),
            keep_unused=True,
        )

    def concat_inputs(self, in_maps):
        return [
            np.concatenate([np.asarray(in_maps[c][name]) for c in range(N_CORES)],
                           axis=0)
            for name in self.in_names
        ]

    def zero_outs(self):
        return [np.zeros((N_CORES * a.shape[0], *a.shape[1:]), a.dtype)
                for a in self.out_avals]

    def run_raw(self, concat_in):
        return self.sharded(*concat_in, *self.zero_outs())

    def run(self, in_maps):
        out_arrs = self.run_raw(self.concat_inputs(in_maps))
        return [
            {name: np.asarray(out_arrs[i]).reshape(N_CORES, *self.out_avals[i].shape)[c]
             for i, name in enumerate(self.out_names)}
            for c in range(N_CORES)
        ]


def _get_exec():
    if "exec" not in _CACHE:
        _CACHE["exec"] = _Exec(_build_nc())
    return _CACHE["exec"]


def _prep_in_maps(hidden_states, gate_w, gate_proj, up_proj, down_proj,
                  s_gate, s_up, s_down):
    f32 = np.float32
    hid = np.ascontiguousarray(hidden_states, dtype=f32)
    hidT = np.ascontiguousarray(hid.transpose(0, 2, 1))
    hidb = hid.astype(BF16)
    gw = np.ascontiguousarray(
        np.asarray(gate_w, f32).reshape(HC, P, E).transpose(1, 0, 2).reshape(P, HC * E))
    ones8 = np.ones((E, 1), f32)
    bselm = np.zeros((E, E * P), f32)
    for b in range(E):
        bselm[b, b * P:(b + 1) * P] = 1.0

    def tile_gu(gT):  # gT [H, X] -> [X//P, P, HC*P]
        X = gT.shape[1]
        return np.ascontiguousarray(
            gT.reshape(HC, P, X // P, P).transpose(2, 1, 0, 3).reshape(X // P, P, HC * P))

    sgT = np.asarray(s_gate, f32).T  # [H, ISH]
    suT = np.asarray(s_up, f32).T
    sgut = np.stack([tile_gu(sgT), tile_gu(suT)]).astype(BF16)
    sdTb = np.ascontiguousarray(np.asarray(s_down, f32).T).astype(BF16)  # [ISH, H]

    gp = np.asarray(gate_proj, f32)
    up = np.asarray(up_proj, f32)
    dn = np.asarray(down_proj, f32)

    in_maps = []
    for c in range(N_CORES):
        gpT = gp[c].T  # [H, I]
        upT = up[c].T
        gut = np.stack([tile_gu(gpT), tile_gu(upT)]).astype(BF16)
        dpTb = np.ascontiguousarray(dn[c].T).astype(BF16)  # [I, H]
        es = np.zeros((E, 1), f32)
        es[c, 0] = 1.0
        in_maps.append({
            "hidT": hidT, "hidb": hidb, "gw": gw, "esel": es,
            "ones8": ones8, "bsel": bselm,
            "gut": gut, "dpTb": dpTb, "sgut": sgut,
            "hshb": hidT[c].astype(BF16), "sdTb": sdTb,
        })
    return in_maps


def _combine(results):
    f32 = np.float32
    comb = np.zeros((B, S, H), f32)
    b_ix = np.arange(B)[:, None]
    for c in range(N_CORES):
        r = results[c]
        w = r["w_out"] * r["scoreso"][:, :, None]
        comb[b_ix, r["idxo"].astype(np.int64)] += w
    shared = np.stack([results[c]["sh_out"] for c in range(N_CORES)])
    return comb.transpose(0, 2, 1) + shared


def kernel(**inputs):
    ex = _get_exec()
    in_maps = _prep_in_maps(**inputs)
    results = ex.run(in_maps)
    return _combine(results).astype(np.float32)



# revision 7
# speedup vs baseline: 2920.4766x; 2920.4766x over previous
"""DeepseekECMoE (expert-choice MoE) Trainium2 kernel, 8-way expert-parallel.

Layout per core c (SPMD, differences only via inputs):
  - gate + exact top-256 routing for batch b=c over ALL experts (f32r gate
    matmul -> softmax -> max8/max_index/match_replace top-k), then a ~16KB
    AllGather exchanges per-core selections so core c obtains (all batches,
    expert c) scores+indices. Routing overlaps the shared-expert matmuls.
  - routed expert c for all 8 batches: dispatch via one-hot matmul (bf16)
    -> expert MLP (bf16 matmuls, exact erf-gelu on ACT) -> unweighted token
    outputs + scores + indices out.
  - shared expert for batch b=c (bf16 matmuls).
Host combines: scatter-add weighted expert outputs, transpose, add shared.
"""
import numpy as np
import ml_dtypes

import concourse.bass as bass
import concourse.tile as tile
from concourse import bacc, mybir
from concourse.bass2jax import install_neuronx_cc_hook, _bass_exec_p, partition_id_tensor

B, S, H, E = 8, 1024, 1024, 8
I, ISH, CAP = 2048, 2048, 256
P = 128
HC, SC, NI, NISH = H // P, S // P, I // P, ISH // P
N_CORES = 8
dt = mybir.dt
BF16 = ml_dtypes.bfloat16

_CACHE: dict = {}


def _build_nc(act_name="Gelu"):
    nc = bacc.Bacc("TRN2", target_bir_lowering=False, debug=False,
                   num_devices=N_CORES)

    # ---- DRAM I/O ----
    hidTc = nc.dram_tensor("hidTc", [H, S], dt.float32r, kind="ExternalInput")
    hidb = nc.dram_tensor("hidb", [B, S, H], dt.bfloat16, kind="ExternalInput")
    gw = nc.dram_tensor("gw", [P, HC * E], dt.float32r, kind="ExternalInput")
    ones8 = nc.dram_tensor("ones8", [E, 1], dt.float32r, kind="ExternalInput")
    onesA = nc.dram_tensor("onesA", [1, E], dt.float32r, kind="ExternalInput")
    esel64 = nc.dram_tensor("esel64", [B * E, E], dt.float32r, kind="ExternalInput")
    bsel = nc.dram_tensor("bsel", [E, E * P], dt.float32r, kind="ExternalInput")
    gut = nc.dram_tensor("gut", [2, NI, P, HC * P], dt.bfloat16, kind="ExternalInput")
    dpTb = nc.dram_tensor("dpTb", [I, H], dt.bfloat16, kind="ExternalInput")
    sgut = nc.dram_tensor("sgut", [2, NISH, P, HC * P], dt.bfloat16, kind="ExternalInput")
    hshb = nc.dram_tensor("hshb", [H, S], dt.bfloat16, kind="ExternalInput")
    sdTb = nc.dram_tensor("sdTb", [ISH, H], dt.bfloat16, kind="ExternalInput")

    # internal bounce buffers for the selection exchange
    selin = nc.dram_tensor("selin", [E, 2 * CAP], dt.float32r)
    selg = nc.dram_tensor("selg", [B * E, 2 * CAP], dt.float32r,
                          addr_space="Shared")

    w_out = nc.dram_tensor("w_out", [B, CAP, H], dt.float32, kind="ExternalOutput")
    scoreso = nc.dram_tensor("scoreso", [B, CAP], dt.float32, kind="ExternalOutput")
    idxo = nc.dram_tensor("idxo", [B, CAP], dt.uint32, kind="ExternalOutput")
    sh_out = nc.dram_tensor("sh_out", [S, H], dt.float32, kind="ExternalOutput")

    AF = mybir.ActivationFunctionType
    ACT = getattr(AF, act_name)
    from contextlib import ExitStack
    with tile.TileContext(nc) as tc:
        with ExitStack() as ctx:
            pool = lambda name, bufs, **kw: ctx.enter_context(
                tc.tile_pool(name=name, bufs=bufs, **kw))
            pconst = pool("consts", 1)
            phtstr = pool("htstr", 3)
            pexp = pool("exp", 1)
            prden = pool("rden", 1)
            proute = pool("route", 1)
            phsh = pool("hsh", 8)
            psw = pool("sw", 4)
            pactsh = pool("actsh", 16)
            pdstr = pool("dstr", 17)
            pactT = pool("actT", 16)
            ptok = pool("tok", 9)
            pM = pool("Mpool", 8)
            phstr = pool("hstr", 9)
            pguw = pool("guw", 4)
            pgel = pool("gel", 2)
            pwo = pool("wo", 3)
            psmall = pool("small", 2)
            pgu = pool("pgu", 2, space="PSUM")
            pdown = pool("pdown", 2, space="PSUM")
            ptokp = pool("ptokp", 2, space="PSUM")
            # ---- constants ----
            t_gw = pconst.tile([P, HC * E], dt.float32r)
            nc.sync.dma_start(t_gw[:], gw[:])
            t_ones8 = pconst.tile([E, 1], dt.float32r)
            nc.sync.dma_start(t_ones8[:], ones8[:])
            t_onesA = pconst.tile([1, E], dt.float32r)
            nc.sync.dma_start(t_onesA[:], onesA[:])
            t_esel64 = pconst.tile([B * E, E], dt.float32r)
            nc.sync.dma_start(t_esel64[:], esel64[:])
            t_bsel = pconst.tile([E, E * P], dt.float32r)
            nc.sync.dma_start(t_bsel[:], bsel[:])
            t_iot = pconst.tile([P, SC], dt.int32)
            nc.gpsimd.iota(t_iot[:], pattern=[[P, SC]], base=0, channel_multiplier=1)
            t_iotf = pconst.tile([P, SC], dt.float32)
            nc.vector.tensor_copy(t_iotf[:], t_iot[:])

            # ---- gate + routing: batch b=c, ALL experts ----
            afftile = proute.tile([E, S], dt.float32)
            exp_b = pexp.tile([E, S], dt.float32r)
            rden = prden.tile([1, S], dt.float32)
            rrec = prden.tile([1, S], dt.float32r)
            for sblk in range(2):
                sl = slice(sblk * 512, (sblk + 1) * 512)
                pl = ptokp.tile([E, 512], dt.float32, tag="ptk", name="pl")
                for hc in range(HC):
                    ht = phtstr.tile([P, 512], dt.float32r)
                    nc.sync.dma_start(ht[:], hidTc[hc * P:(hc + 1) * P, sl])
                    nc.tensor.matmul(pl[:], t_gw[:, hc * E:(hc + 1) * E],
                                     ht[:], start=(hc == 0), stop=(hc == HC - 1))
                nc.scalar.activation(exp_b[:, sl], pl[:], AF.Exp)
                pden = ptokp.tile([1, 512], dt.float32, tag="ptk", name="pden")
                nc.tensor.matmul(pden[:], t_ones8[:], exp_b[:, sl],
                                 start=True, stop=True)
                nc.scalar.copy(rden[:, sl], pden[:])
            with nc.allow_low_precision(reason="f32r output is f32 bits"):
                nc.vector.reciprocal(rrec[:], rden[:])
            for sblk in range(2):
                sl = slice(sblk * 512, (sblk + 1) * 512)
                pbc = ptokp.tile([E, 512], dt.float32, tag="ptk", name="pbc")
                nc.tensor.matmul(pbc[:], t_onesA[:], rrec[:, sl],
                                 start=True, stop=True)
                nc.vector.tensor_mul(afftile[:, sl], exp_b[:, sl], pbc[:])

            t_scores = proute.tile([E, CAP], dt.float32)
            t_idxu = proute.tile([E, CAP], dt.uint32)
            for i in range(CAP // 8):
                sc8 = t_scores[:, i * 8:(i + 1) * 8]
                nc.vector.max(sc8, afftile[:])
                nc.vector.max_index(t_idxu[:, i * 8:(i + 1) * 8], sc8, afftile[:])
                nc.vector.match_replace(afftile[:], sc8, afftile[:], -1e30)
            t_idxf = proute.tile([E, CAP], dt.float32)
            nc.vector.tensor_copy(t_idxf[:], t_idxu[:])

            # ---- exchange: [scores | idx] AllGather, extract expert c ----
            t_sel = proute.tile([E, 2 * CAP], dt.float32r)
            nc.vector.tensor_copy(t_sel[:, :CAP], t_scores[:])
            nc.vector.tensor_copy(t_sel[:, CAP:], t_idxf[:])
            nc.sync.dma_start(selin[:], t_sel[:])
            nc.gpsimd.collective_compute(
                "AllGather", mybir.AluOpType.bypass,
                replica_groups=[list(range(N_CORES))],
                ins=[selin[:].opt()], outs=[selg[:].opt()])
            t_g = proute.tile([B * E, 2 * CAP], dt.float32r)
            nc.sync.dma_start(t_g[:], selg[:])
            pex = ptokp.tile([E, 2 * CAP], dt.float32, tag="ptk", name="pex")
            nc.tensor.matmul(pex[:], t_esel64[:], t_g[:], start=True, stop=True)
            sco = psmall.tile([E, CAP], dt.float32, tag="sco", name="sco")
            nc.scalar.copy(sco[:], pex[:, :CAP])
            nc.sync.dma_start(scoreso[:], sco[:])
            t_idxf2 = proute.tile([E, CAP], dt.float32)
            nc.vector.tensor_copy(t_idxf2[:], pex[:, CAP:])
            t_idxu2 = proute.tile([E, CAP], dt.uint32)
            nc.vector.tensor_copy(t_idxu2[:], t_idxf2[:])
            nc.sync.dma_start(idxo[:], t_idxu2[:])
            t_idxfr = proute.tile([E, CAP], dt.float32r)
            nc.vector.tensor_copy(t_idxfr[:], t_idxf2[:])

            # ---- shared expert (batch c fed via hshb) ----
            hsh = []
            for hc in range(HC):
                t = phsh.tile([P, S], dt.bfloat16, tag="hsh", name="hsh")
                nc.sync.dma_start(t[:], hshb[hc * P:(hc + 1) * P, :])
                hsh.append(t)
            actsh = []
            for i in range(NISH):
                sg = psw.tile([P, HC * P], dt.bfloat16, bufs=2)
                nc.sync.dma_start(sg[:], sgut[0, i])
                su = psw.tile([P, HC * P], dt.bfloat16, bufs=2)
                nc.sync.dma_start(su[:], sgut[1, i])
                a = pactsh.tile([P, S], dt.bfloat16)
                for sblk in range(2):
                    pg = pgu.tile([P, 512], dt.float32, tag="pg", name="pg", bufs=2)
                    for hc in range(HC):
                        nc.tensor.matmul(pg[:], sg[:, hc * P:(hc + 1) * P],
                                         hsh[hc][:, sblk * 512:(sblk + 1) * 512],
                                         start=(hc == 0), stop=(hc == HC - 1))
                    pu = pgu.tile([P, 512], dt.float32, tag="pu", name="pu", bufs=2)
                    for hc in range(HC):
                        nc.tensor.matmul(pu[:], su[:, hc * P:(hc + 1) * P],
                                         hsh[hc][:, sblk * 512:(sblk + 1) * 512],
                                         start=(hc == 0), stop=(hc == HC - 1))
                    gel = pgel.tile([P, 512], dt.float32)
                    nc.scalar.activation(gel[:], pg[:], ACT)
                    nc.vector.tensor_mul(a[:, sblk * 512:(sblk + 1) * 512],
                                         gel[:], pu[:])
                actsh.append(a)
            sdt = []
            for ic in range(NISH):
                t = pdstr.tile([P, H], dt.bfloat16, tag="dstr", name="dstr")
                nc.sync.dma_start(t[:], sdTb[ic * P:(ic + 1) * P, :])
                sdt.append(t)
            for sblk in range(SC):
                for hh in range(2):
                    pd = pdown.tile([P, 512], dt.float32)
                    for ic in range(NISH):
                        nc.tensor.matmul(pd[:],
                                         actsh[ic][:, sblk * P:(sblk + 1) * P],
                                         sdt[ic][:, hh * 512:(hh + 1) * 512],
                                         start=(ic == 0), stop=(ic == NISH - 1))
                    sho = pwo.tile([P, 512], dt.float32, tag="wo", name="wo")
                    nc.scalar.copy(sho[:], pd[:])
                    nc.sync.dma_start(
                        sh_out[sblk * P:(sblk + 1) * P, hh * 512:(hh + 1) * 512],
                        sho[:])

            # ---- routed expert, batch pairs ----
            for pair in range(B // 2):
                b0 = 2 * pair
                tokT = []
                for hc in range(HC):
                    tokT.append(ptok.tile([P, 2 * CAP], dt.bfloat16, tag="tokT", name="tokT"))
                for bi in range(2):
                    b = b0 + bi
                    pib = ptokp.tile([P, CAP], dt.float32, tag="ptk", name="pib")
                    nc.tensor.matmul(pib[:], t_bsel[:, b * P:(b + 1) * P],
                                     t_idxfr[:], start=True, stop=True)
                    idxB = psmall.tile([P, CAP], dt.float32)
                    nc.vector.tensor_copy(idxB[:], pib[:])
                    Ms = []
                    for sc in range(SC):
                        m = pM.tile([P, CAP], dt.bfloat16, tag="M", name="M")
                        nc.vector.tensor_scalar(m[:], idxB[:], t_iotf[:, sc:sc + 1],
                                                None, mybir.AluOpType.is_equal)
                        Ms.append(m)
                    hh_tiles = []
                    for sc in range(SC):
                        t = phstr.tile([P, H], dt.bfloat16, tag="hstr", name="hstr")
                        nc.sync.dma_start(t[:], hidb[b, sc * P:(sc + 1) * P, :])
                        hh_tiles.append(t)
                    for hblk in range(HC):
                        pt = ptokp.tile([P, CAP], dt.float32, tag="ptk", name="pt")
                        for sc in range(SC):
                            nc.tensor.matmul(pt[:],
                                             hh_tiles[sc][:, hblk * P:(hblk + 1) * P],
                                             Ms[sc][:],
                                             start=(sc == 0), stop=(sc == SC - 1))
                        nc.vector.tensor_copy(
                            tokT[hblk][:, bi * CAP:(bi + 1) * CAP], pt[:])

                actT = []
                for i in range(NI):
                    sg = pguw.tile([P, HC * P], dt.bfloat16, bufs=2)
                    nc.sync.dma_start(sg[:], gut[0, i])
                    su = pguw.tile([P, HC * P], dt.bfloat16, bufs=2)
                    nc.sync.dma_start(su[:], gut[1, i])
                    pg = pgu.tile([P, 2 * CAP], dt.float32, tag="pg", name="pg", bufs=2)
                    for hc in range(HC):
                        nc.tensor.matmul(pg[:], sg[:, hc * P:(hc + 1) * P],
                                         tokT[hc][:],
                                         start=(hc == 0), stop=(hc == HC - 1))
                    pu = pgu.tile([P, 2 * CAP], dt.float32, tag="pu", name="pu", bufs=2)
                    for hc in range(HC):
                        nc.tensor.matmul(pu[:], su[:, hc * P:(hc + 1) * P],
                                         tokT[hc][:],
                                         start=(hc == 0), stop=(hc == HC - 1))
                    gel = pgel.tile([P, 2 * CAP], dt.float32)
                    nc.scalar.activation(gel[:], pg[:], ACT)
                    a = pactT.tile([P, 2 * CAP], dt.bfloat16)
                    nc.vector.tensor_mul(a[:], gel[:], pu[:])
                    actT.append(a)

                dpt = []
                for ic in range(NI):
                    t = pdstr.tile([P, H], dt.bfloat16, tag="dstr", name="dstr")
                    nc.sync.dma_start(t[:], dpTb[ic * P:(ic + 1) * P, :])
                    dpt.append(t)
                for tb in range(4):
                    b = b0 + tb // 2
                    rblk = tb % 2
                    for hh in range(2):
                        pd = pdown.tile([P, 512], dt.float32)
                        for ic in range(NI):
                            nc.tensor.matmul(pd[:],
                                             actT[ic][:, tb * P:(tb + 1) * P],
                                             dpt[ic][:, hh * 512:(hh + 1) * 512],
                                             start=(ic == 0), stop=(ic == NI - 1))
                        wo = pwo.tile([P, 512], dt.float32, tag="wo", name="wo")
                        nc.scalar.copy(wo[:], pd[:])
                        nc.sync.dma_start(
                            w_out[b, rblk * P:(rblk + 1) * P,
                                  hh * 512:(hh + 1) * 512], wo[:])

    nc.compile()
    return nc


class _Exec:
    """Cached multi-core PJRT executor (mirrors bass2jax.run_bass_via_pjrt).

    Output buffers are created device-side (jnp.zeros inside the jitted body)
    so repeat calls transfer no host data."""

    def __init__(self, nc):
        import jax
        import jax.numpy as jnp
        from jax.sharding import Mesh, PartitionSpec
        from jax.experimental.shard_map import shard_map

        install_neuronx_cc_hook()
        self.nc = nc
        in_names, out_names, out_avals = [], [], []
        partition_name = (nc.partition_id_tensor.name
                          if nc.partition_id_tensor else None)
        for alloc in nc.m.functions[0].allocations:
            if not isinstance(alloc, mybir.MemoryLocationSet):
                continue
            if alloc.kind not in ("ExternalInput", "ExternalOutput"):
                continue
            name = alloc.memorylocations[0].name
            if alloc.kind == "ExternalInput":
                if name != partition_name:
                    in_names.append(name)
            elif alloc.kind == "ExternalOutput":
                out_names.append(name)
                out_avals.append(jax.core.ShapedArray(
                    tuple(alloc.tensor_shape), mybir.dt.np(alloc.dtype)))
        self.in_names, self.out_names, self.out_avals = in_names, out_names, out_avals
        self.partition_name = partition_name
        n_params = len(in_names)
        all_in_names = list(in_names) + list(out_names)
        if partition_name is not None:
            all_in_names.append(partition_name)

        def _body(*args):
            operands = list(args)
            if partition_name is not None:
                operands.append(partition_id_tensor())
            outs = _bass_exec_p.bind(
                *operands,
                out_avals=tuple(out_avals),
                in_names=tuple(all_in_names),
                out_names=tuple(out_names),
                lowering_input_output_aliases=(),
                sim_require_finite=True,
                sim_require_nnan=True,
                nc=nc,
            )
            return tuple(outs)

        devices = jax.devices()[:N_CORES]
        mesh = Mesh(np.asarray(devices), ("core",))
        in_specs = (PartitionSpec("core"),) * (n_params + len(out_names))
        out_specs = (PartitionSpec("core"),) * len(out_names)
        self.sharded = jax.jit(
            shard_map(_body, mesh=mesh, in_specs=in_specs, out_specs=out_specs,
                      check_rep=False),
            keep_unused=True,
        )
        # device-resident zero output operands, staged once and reused
        # (not donated, so they survive across calls)
        self._dev_zeros = None

    def concat_inputs(self, in_maps):
        return [
            np.concatenate([np.asarray(in_maps[c][name]) for c in range(N_CORES)],
                           axis=0)
            for name in self.in_names
        ]

    def zero_outs(self):
        return [np.zeros((N_CORES * a.shape[0], *a.shape[1:]), a.dtype)
                for a in self.out_avals]

    def run_raw(self, concat_in):
        import jax
        if self._dev_zeros is None:
            self._dev_zeros = [jax.device_put(z) for z in self.zero_outs()]
        return self.sharded(*concat_in, *self._dev_zeros)

    def run(self, in_maps):
        out_arrs = self.run_raw(self.concat_inputs(in_maps))
        return [
            {name: np.asarray(out_arrs[i]).reshape(N_CORES, *self.out_avals[i].shape)[c]
             for i, name in enumerate(self.out_names)}
            for c in range(N_CORES)
        ]


def _get_exec():
    if "exec" not in _CACHE:
        _CACHE["exec"] = _Exec(_build_nc())
    return _CACHE["exec"]


def _prep_in_maps(hidden_states, gate_w, gate_proj, up_proj, down_proj,
                  s_gate, s_up, s_down):
    f32 = np.float32
    hid = np.ascontiguousarray(hidden_states, dtype=f32)
    hidT = np.ascontiguousarray(hid.transpose(0, 2, 1))
    hidb = hid.astype(BF16)
    gw = np.ascontiguousarray(
        np.asarray(gate_w, f32).reshape(HC, P, E).transpose(1, 0, 2).reshape(P, HC * E))
    ones8 = np.ones((E, 1), f32)
    onesA = np.ones((1, E), f32)
    bselm = np.zeros((E, E * P), f32)
    for b in range(E):
        bselm[b, b * P:(b + 1) * P] = 1.0

    def tile_gu(gT):  # gT [H, X] -> [X//P, P, HC*P]
        X = gT.shape[1]
        return np.ascontiguousarray(
            gT.reshape(HC, P, X // P, P).transpose(2, 1, 0, 3).reshape(X // P, P, HC * P))

    sgT = np.asarray(s_gate, f32).T  # [H, ISH]
    suT = np.asarray(s_up, f32).T
    sgut = np.stack([tile_gu(sgT), tile_gu(suT)]).astype(BF16)
    sdTb = np.ascontiguousarray(np.asarray(s_down, f32).T).astype(BF16)  # [ISH, H]

    gp = np.asarray(gate_proj, f32)
    up = np.asarray(up_proj, f32)
    dn = np.asarray(down_proj, f32)

    in_maps = []
    for c in range(N_CORES):
        gpT = gp[c].T  # [H, I]
        upT = up[c].T
        gut = np.stack([tile_gu(gpT), tile_gu(upT)]).astype(BF16)
        dpTb = np.ascontiguousarray(dn[c].T).astype(BF16)  # [I, H]
        esel64 = np.zeros((B * E, E), f32)
        for b in range(B):
            esel64[b * E + c, b] = 1.0
        in_maps.append({
            "hidTc": hidT[c], "hidb": hidb, "gw": gw,
            "ones8": ones8, "onesA": onesA, "esel64": esel64, "bsel": bselm,
            "gut": gut, "dpTb": dpTb, "sgut": sgut,
            "hshb": hidT[c].astype(BF16), "sdTb": sdTb,
        })
    return in_maps


def _combine(results):
    f32 = np.float32
    comb = np.zeros((B, S, H), f32)
    b_ix = np.arange(B)[:, None]
    for c in range(N_CORES):
        r = results[c]
        w = r["w_out"] * r["scoreso"][:, :, None]
        comb[b_ix, r["idxo"].astype(np.int64)] += w
    shared = np.stack([results[c]["sh_out"] for c in range(N_CORES)])
    return comb.transpose(0, 2, 1) + shared


def kernel(**inputs):
    ex = _get_exec()
    in_maps = _prep_in_maps(**inputs)
    results = ex.run(in_maps)
    return _combine(results).astype(np.float32)
